# revision 34
# baseline (speedup 1.0000x reference)
"""Trainium2 Bass kernel for multi-head attention (B=4, N=4096, D=384, H=6).

Sharding: 8 cores = 4 batches x 2 head-groups (3 heads each).
Host pre-transposes and pre-casts all inputs to bf16 in the layouts the
kernel wants (xT[d, n], wT[d, e], wpT[c, e]), so the device does zero
transposes/casts in the prefix. Each core computes, for its
(batch, head-group):
    qkv = x @ W_g^T            (bf16 matmuls, fp32 PSUM accumulate)
    per head: scoresT = (k q^T)^T laid out [keys(m) partition, queries(n) free]
    e = exp(scoresT * scale)   (no max subtraction; scores ~ N(0,1) after scale)
    PV with an appended ones-column in V: row 64 of the PSUM accumulator is
    the softmax denominator S[n].
    normalized headsT -> partial projection yT_g
Host sums the two head-group partials per batch, transposes, adds bias.
"""

import os
import sys

import numpy as np

B, NSEQ, D = 4, 4096, 384
H, DH = 6, 64
HPC = 3  # heads per core
NCORES = 8
GCOLS = HPC * DH  # 192
SCALE = float(DH) ** -0.5


def _ensure_path():
    p = "/opt/trn_rl_repo"
    if os.path.isdir(p) and p not in sys.path:
        sys.path.insert(0, p)


def build_nc(n_seq=NSEQ):
    """Build the single-core Bass program (SPMD across 8 cores)."""
    _ensure_path()
    from contextlib import ExitStack

    import concourse.bacc as bacc
    import concourse.tile as tile
    from concourse import mybir

    f32 = mybir.dt.float32
    bf16 = mybir.dt.bfloat16
    i16 = mybir.dt.int16
    EXP = mybir.ActivationFunctionType.Exp

    # DVE fast-exp: exp(s*SCALE) ~= bitcast_bf16(int16(s*EA + EB1))
    #             + bitcast_bf16(int16(s*EA + EB2))
    # Two phase-shifted piecewise-linear 2^y constructions summed: max rel
    # err 1.2%, rms 0.7% (vs 4.2%/1.8% for the single-term Schraudolph).
    LOG2E = 1.4426950408889634
    EA = SCALE * LOG2E * 128.0
    EB1 = 16150.35
    EDELTA = -63.0
    # pairs per 16-pair head block whose exp runs on the DVE instead of ACT
    OFF_PAIRS = frozenset({1, 3, 6, 9, 11})

    assert n_seq % 512 == 0
    nt = n_seq // 128
    nch = n_seq // 512

    nc = bacc.Bacc("TRN2", target_bir_lowering=False, debug=False)

    # Host-prepped transposed bf16 inputs.
    xT_d = nc.dram_tensor("xT", [128, 3 * n_seq], bf16, kind="ExternalInput")
    wT_d = nc.dram_tensor("wT", [128, 3 * 3 * GCOLS], bf16, kind="ExternalInput")
    u_d = nc.dram_tensor("uT", [3 * 64, n_seq], bf16, kind="ExternalOutput")
    s_d = nc.dram_tensor("sT", [3, n_seq], f32, kind="ExternalOutput")

    with tile.TileContext(nc) as tc, ExitStack() as ctx:
        constp = ctx.enter_context(tc.tile_pool(name="const", bufs=1))
        persist = ctx.enter_context(tc.tile_pool(name="persist", bufs=1))
        hpool = ctx.enter_context(tc.tile_pool(name="headsp", bufs=2))
        ypool = ctx.enter_context(tc.tile_pool(name="youtp", bufs=3))
        epool = ctx.enter_context(tc.tile_pool(name="epool", bufs=5))
        rcpool = ctx.enter_context(tc.tile_pool(name="rcp", bufs=2))

        # Transposed weights straight from DRAM: wT[p, d3, e] = W_g[e, 128*d3+p]
        wT = persist.tile([128, 3, 3 * GCOLS], bf16)
        nc.sync.dma_start(out=wT.rearrange("p a b -> p (a b)"), in_=wT_d[:, :])
        # xT[p, d3, n] = x[n, 128*d3 + p]; DMA'd in 4 n-groups so the first
        # qkv chunk can start after ~1/4 of the transfer.
        xT = persist.tile([128, 3, n_seq], bf16)
        ng = n_seq // 4
        for g in range(4):
            nc.sync.dma_start(
                out=xT[:, :, ng * g : ng * (g + 1)],
                in_=xT_d[:, 3 * ng * g : 3 * ng * (g + 1)].rearrange(
                    "p (a b) -> p a b", a=3
                ),
            )

        # Per-head q/k in transposed layout [c, n], bf16, REPLICATED across
        # both 64-partition halves so score matmuls can row-pack in pairs.
        t_q0 = persist.tile([128, n_seq], bf16)
        t_q1 = persist.tile([128, n_seq], bf16)
        t_q2 = persist.tile([128, n_seq], bf16)
        t_k0 = persist.tile([128, n_seq], bf16)
        t_k1 = persist.tile([128, n_seq], bf16)
        t_k2 = persist.tile([128, n_seq], bf16)
        # V in natural layout per n-tile, per head, with a ones column (col 64)
        v_sb = persist.tile([128, nt, 3, 65], bf16)
        ones_nt = constp.tile([128, nt * 3], f32)
        nc.vector.memset(ones_nt, 1.0)
        nc.vector.tensor_copy(
            v_sb[:, :, :, 64:65].rearrange("p t h o -> p (t h o)"), ones_nt
        )

        with (
            tc.tile_pool(name="pqk", bufs=3, space="PSUM") as pqk,
            tc.tile_pool(name="pvp", bufs=2, space="PSUM") as pvp,
        ):
            # ---- qkv per chunk (all bf16) ----
            for j in range(nch):
                # q/k (transposed layout); host lays the W rows out as
                # [q01 (128) | k01 (128) | q2,k2 (128) | v (192)] so each
                # M=128 group is a contiguous lhsT slice. Each 64-row result
                # is copied to the matching partition half of its replicated
                # destination tile.
                for c0, dests in (
                    (0, ((t_q0, 0, 0), (t_q1, 64, 64))),
                    (128, ((t_k0, 0, 0), (t_k1, 64, 64))),
                    (256, ((t_q2, 0, 0), (t_k2, 64, 64))),
                ):
                    ps = pqk.tile([128, 512], f32, tag="pqk")
                    for d3 in range(3):
                        nc.tensor.matmul(
                            ps,
                            wT[:, d3, c0 : c0 + 128],
                            xT[:, d3, 512 * j : 512 * (j + 1)],
                            start=(d3 == 0),
                            stop=(d3 == 2),
                        )
                    for dest, sp, dp in dests:
                        dst_ap = dest[dp : dp + 64, 512 * j : 512 * (j + 1)]
                        if c0 < 256:
                            nc.scalar.copy(dst_ap, ps[sp : sp + 64, :])
                        else:
                            nc.vector.tensor_copy(dst_ap, ps[sp : sp + 64, :])

                # v in natural layout
                for tt in range(4 * j, 4 * j + 4):
                    ps = pvp.tile([128, 192], f32, tag="pv")
                    for d3 in range(3):
                        nc.tensor.matmul(
                            ps,
                            xT[:, d3, 128 * tt : 128 * (tt + 1)],
                            wT[:, d3, 384:576],
                            start=(d3 == 0),
                            stop=(d3 == 2),
                        )
                    nc.vector.tensor_copy(
                        v_sb[:, tt, :, 0:64],
                        ps.rearrange("p (h c) -> p h c", h=3),
                    )

            # replicate each q/k head across the other partition half
            for tq in (t_q0, t_k0):
                nc.sync.dma_start(out=tq[64:128, :], in_=tq[0:64, :])
            for tq in (t_q1, t_k1, t_k2):
                nc.sync.dma_start(out=tq[0:64, :], in_=tq[64:128, :])
            nc.sync.dma_start(out=t_q2[64:128, :], in_=t_q2[0:64, :])

        # ---- attention + projection ----
        # Pipelined: scores for t-pairs into a 2-bank PSUM tile, one exp ACT
        # per pair, PV accumulation trailing. Each head's normalize +
        # projection tail is deferred into the next head's compute so the PE
        # never drains (keeps the HAM clock un-throttled). The tail uses its
        # own 1-bank PSUM pool so it never steals a scores buffer.
        heads_qk = [(t_k0, t_q0), (t_k1, t_q1), (t_k2, t_q2)]
        with (
            tc.tile_pool(name="psc", bufs=2, space="PSUM") as psc,
            tc.tile_pool(name="pop", bufs=2, space="PSUM") as pop,
        ):

            def make_tail(j, h, po_t, headsT):
                # Stash the UNNORMALIZED bf16 head output and its softmax
                # denominator row straight to DRAM; the host does both the
                # normalization and the output projection (one sgemm).
                s_sb = rcpool.tile([65, 512], f32, tag="s_sb", name="s_sb")
                u_sb = rcpool.tile([64, 512], bf16, tag="u_sb", name="u_sb")

                def stash():
                    nc.vector.tensor_copy(u_sb, po_t[0:64, :])
                    nc.scalar.copy(s_sb[64:65, :], po_t[64:65, :])
                    nc.sync.dma_start(
                        out=s_d[h : h + 1, 512 * j : 512 * (j + 1)],
                        in_=s_sb[64:65, :],
                    )

                def ship():
                    nc.sync.dma_start(
                        out=u_d[64 * h : 64 * (h + 1), 512 * j : 512 * (j + 1)],
                        in_=u_sb,
                    )

                # (slot, fn): dispatched once the next head's pair index
                # reaches `slot`; short PE->DVE chains only, so nothing in
                # any engine queue waits long.
                return [(3, stash), (5, ship)]

            PV_DELAY = min(3, nt // 2 - 1)  # pairs the PV matmuls trail by
            OFF_PV_DELAY = PV_DELAY + 3  # extra trail for DVE fast-exp pairs

            # PV queue survives head/chunk boundaries so the PE stream never
            # bunches up (bunched PVs stall the ACT and re-throttle the HAM).
            pv_queue = []
            stop_done = set()  # (po id) whose stop-flagged PV has been emitted
            # deferred 2nd-term + sum ops of DVE fast-exp pairs
            off_queue = []

            def flush_off():
                e_t, ea_t = off_queue.pop(0)
                eb_t = epool.tile([128, 2, 512], bf16, tag="eb", name="eb_t", bufs=6)
                nc.vector.tensor_scalar_add(
                    eb_t.bitcast(i16), ea_t.bitcast(i16), EDELTA
                )
                nc.vector.tensor_add(e_t, ea_t, eb_t)

            def emit_pv(ent):
                _, po_q, h_q, t0, e_t = ent
                for s in range(2):
                    t = t0 + s
                    nc.tensor.matmul(
                        po_q,
                        v_sb[:, t, h_q, :],
                        e_t[:, s, :],
                        start=(t == 0),
                        stop=(t == nt - 1),
                    )
                if t0 == nt - 2:
                    stop_done.add(id(po_q))

            def flush_pv():
                emit_pv(pv_queue.pop(0))

            def pump_pv(g_now):
                # Flush the oldest sufficiently-aged entry; fast-exp pairs
                # age longer so their PV never waits on the deferred sum.
                # (Accumulation order between start and stop is free.)
                for i in range(min(6, len(pv_queue))):
                    g_emit, _, _, t0, e_t = pv_queue[i]
                    if i > 0 and t0 == nt - 2:
                        # never emit the stop-flagged pair out of order
                        return
                    need = OFF_PV_DELAY if (t0 // 2) in OFF_PAIRS else PV_DELAY
                    if g_now - g_emit >= need:
                        emit_pv(pv_queue.pop(i))
                        return

            pending_tail = []
            pending_po_id = None
            for j in range(nch):
                headsT = None
                for h in range(3):
                    kt, qt = heads_qk[h]
                    po_t = pop.tile([65, 512], f32, tag="po")
                    g_head = (3 * j + h) * 16
                    for tp in range(nt // 2):
                        pump_pv(g_head + tp)
                        if off_queue and tp not in OFF_PAIRS:
                            flush_off()
                        t0 = 2 * tp
                        ps = psc.tile([128, 2, 512], f32, tag="ps")
                        for s in range(2):
                            # row-packed pair: s=0 on partitions 0-63,
                            # s=1 on partitions 64-127 (concurrent on PE)
                            t = t0 + s
                            pb = 64 * s
                            nc.tensor.matmul(
                                ps[:, s, :],
                                kt[pb : pb + 64, 128 * t : 128 * (t + 1)],
                                qt[pb : pb + 64, 512 * j : 512 * (j + 1)],
                            )
                        e_t = epool.tile([128, 2, 512], bf16, tag="e_t", bufs=10)
                        if tp in OFF_PAIRS:
                            # DVE fast-exp: affine+round-to-int16 writes a
                            # phase-shifted PL approximation of exp(s*SCALE)
                            # in bf16 bit-space; a second term (bits-63, an
                            # exact int shift that never re-reads PSUM) is
                            # added later so the scores buffer frees as fast
                            # as an ACT exp. Summed max rel err: 1.3%.
                            ea_t = epool.tile([128, 2, 512], bf16, tag="ea", bufs=6)
                            nc.vector.tensor_scalar(
                                out=ea_t.bitcast(i16),
                                in0=ps[:],
                                scalar1=EA,
                                scalar2=EB1,
                                op0=mybir.AluOpType.mult,
                                op1=mybir.AluOpType.add,
                            )
                            off_queue.append((e_t, ea_t))
                        else:
                            nc.scalar.activation(e_t, ps[:], EXP, scale=SCALE)
                        pv_queue.append((g_head + tp, po_t, h, t0, e_t))
                        # tail parts only once their head's accumulation has
                        # fully been emitted (stop PV in the PE stream)
                        while (
                            pending_tail
                            and tp >= pending_tail[0][0]
                            and pending_po_id in stop_done
                        ):
                            pending_tail.pop(0)[1]()
                        if len(pv_queue) > 5:
                            pump_pv(g_head + tp)
                    while pending_tail:
                        if pending_po_id not in stop_done:
                            pump_pv(10 ** 9)
                            continue
                        pending_tail.pop(0)[1]()
                    pending_tail = make_tail(j, h, po_t, headsT)
                    pending_po_id = id(po_t)
            while off_queue:
                flush_off()
            while pv_queue:
                flush_pv()
            for _, part in pending_tail:
                part()

    nc.compile()
    return nc


def shard_inputs(x, W_qkv, W_proj):
    """Full inputs -> per-core input maps (host transposes + bf16 casts)."""
    import ml_dtypes

    bf16 = ml_dtypes.bfloat16
    x = np.asarray(x, dtype=np.float32)
    W_qkv = np.asarray(W_qkv, dtype=np.float32)
    W_proj = np.asarray(W_proj, dtype=np.float32)
    d = W_qkv.shape[1]
    n_seq = x.shape[1]
    in_maps = []
    for c in range(NCORES):
        b, g = divmod(c, 2)
        sl = slice(GCOLS * g, GCOLS * (g + 1))
        w_q = W_qkv[0 * d :][sl]  # [192, 384]
        w_k = W_qkv[1 * d :][sl]
        w_v = W_qkv[2 * d :][sl]
        # Row order [q01 | k01 | q2,k2 | v] so each qkv matmul group is a
        # contiguous 128-col lhsT slice of wT.
        w_g = np.concatenate(
            [w_q[0:128], w_k[0:128], w_q[128:192], w_k[128:192], w_v], axis=0
        )
        # wT[p, d3, e] = W_g[e, 128*d3 + p]  -> [128, 3*576]
        wT = np.ascontiguousarray(
            w_g.T.reshape(3, 128, 3 * GCOLS).transpose(1, 0, 2).reshape(128, -1)
        ).astype(bf16)
        # xT[p, d3, n] = x[b][n, 128*d3 + p] -> [128, 3*n_seq], stored in 4
        # contiguous n-groups of [3, n/4] to match the kernel's grouped DMA.
        xT = (
            x[b].T.reshape(3, 128, n_seq)
            .transpose(1, 0, 2)  # [128, 3, n]
            .reshape(128, 3, 4, n_seq // 4)
            .transpose(0, 2, 1, 3)  # [128, 4, 3, n/4]
            .reshape(128, -1)
        )
        xT = np.ascontiguousarray(xT).astype(bf16)
        in_maps.append({"xT": xT, "wT": wT})
    return in_maps


def combine_outputs(results, W_proj, b_proj, n_seq=NSEQ):
    """Per-core (uT, sT) partials -> full [B, N, D] output.

    uT: [192, n_seq] unnormalized bf16 head outputs; sT: [3, n_seq] softmax
    denominators. Host normalizes, projects (sgemm) and sums head-groups.
    """
    W_proj = np.asarray(W_proj, dtype=np.float32)
    b_proj = np.asarray(b_proj, dtype=np.float32)
    y = np.empty((B, n_seq, D), dtype=np.float32)
    for b in range(B):
        acc = np.zeros((D, n_seq), dtype=np.float32)
        for g in range(2):
            r = results[2 * b + g]
            u = r["uT"].astype(np.float32).reshape(3, 64, n_seq)
            u /= r["sT"][:, None, :]
            wp_g = W_proj[:, GCOLS * g : GCOLS * (g + 1)]  # [D, 192]
            acc += wp_g @ u.reshape(192, n_seq)
        y[b] = acc.T + b_proj
    return y


_NC_CACHE = {}


def kernel(**inputs):
    _ensure_path()
    from concourse.bass_utils import run_bass_kernel_spmd

    x = np.asarray(inputs["x"], dtype=np.float32)
    W_qkv = np.asarray(inputs["W_qkv"], dtype=np.float32)
    W_proj = np.asarray(inputs["W_proj"], dtype=np.float32)
    b_proj = np.asarray(inputs["b_proj"], dtype=np.float32)

    n_seq = x.shape[1]
    if n_seq not in _NC_CACHE:
        _NC_CACHE[n_seq] = build_nc(n_seq)
    nc = _NC_CACHE[n_seq]

    in_maps = shard_inputs(x, W_qkv, W_proj)
    res = run_bass_kernel_spmd(nc, in_maps, core_ids=list(range(NCORES)))
    return combine_outputs(res.results, W_proj, b_proj, n_seq)


if __name__ == "__main__":
    rng = np.random.default_rng(0)
    n = 512
    x = rng.standard_normal((B, n, D), dtype=np.float32)
    wq = (rng.standard_normal((3 * D, D), dtype=np.float32) / np.sqrt(D)).astype(np.float32)
    wp = (rng.standard_normal((D, D), dtype=np.float32) / np.sqrt(D)).astype(np.float32)
    bp = np.zeros(D, np.float32)
    out = kernel(x=x, W_qkv=wq, W_proj=wp, b_proj=bp)
    print(out.shape, out.dtype)


# revision 35
# speedup vs baseline: 1.1017x; 1.1017x over previous
"""Trainium2 Bass kernel for multi-head attention (B=4, N=4096, D=384, H=6).

Sharding: 8 cores = 4 batches x 2 head-groups (3 heads each).
Host pre-transposes and pre-casts all inputs to bf16 in the layouts the
kernel wants (xT[d, n], wT[d, e], wpT[c, e]), so the device does zero
transposes/casts in the prefix. Each core computes, for its
(batch, head-group):
    qkv = x @ W_g^T            (bf16 matmuls, fp32 PSUM accumulate)
    per head: scoresT = (k q^T)^T laid out [keys(m) partition, queries(n) free]
    e = exp(scoresT * scale)   (no max subtraction; scores ~ N(0,1) after scale)
    PV with an appended ones-column in V: row 64 of the PSUM accumulator is
    the softmax denominator S[n].
    normalized headsT -> partial projection yT_g
Host sums the two head-group partials per batch, transposes, adds bias.
"""

import os
import sys

import numpy as np

B, NSEQ, D = 4, 4096, 384
H, DH = 6, 64
HPC = 3  # heads per core
NCORES = 8
GCOLS = HPC * DH  # 192
SCALE = float(DH) ** -0.5


def _ensure_path():
    p = "/opt/trn_rl_repo"
    if os.path.isdir(p) and p not in sys.path:
        sys.path.insert(0, p)


def build_nc(n_seq=NSEQ):
    """Build the single-core Bass program (SPMD across 8 cores)."""
    _ensure_path()
    from contextlib import ExitStack

    import concourse.bacc as bacc
    import concourse.tile as tile
    from concourse import mybir

    f32 = mybir.dt.float32
    bf16 = mybir.dt.bfloat16
    i16 = mybir.dt.int16
    EXP = mybir.ActivationFunctionType.Exp

    # DVE fast-exp: exp(s*SCALE) ~= bitcast_bf16(int16(s*EA + EB1))
    #             + bitcast_bf16(int16(s*EA + EB2))
    # Two phase-shifted piecewise-linear 2^y constructions summed: max rel
    # err 1.2%, rms 0.7% (vs 4.2%/1.8% for the single-term Schraudolph).
    LOG2E = 1.4426950408889634
    EA = SCALE * LOG2E * 128.0
    EB1 = 16150.35
    EDELTA = -63.0
    # pairs per 16-pair head block whose exp runs on the DVE instead of ACT
    OFF_PAIRS = frozenset({2, 5, 8, 11})

    assert n_seq % 512 == 0
    nt = n_seq // 128
    nch = n_seq // 512

    nc = bacc.Bacc("TRN2", target_bir_lowering=False, debug=False)

    # Host-prepped transposed bf16 inputs.
    xT_d = nc.dram_tensor("xT", [128, 3 * n_seq], bf16, kind="ExternalInput")
    wT_d = nc.dram_tensor("wT", [128, 3 * 3 * GCOLS], bf16, kind="ExternalInput")
    u_d = nc.dram_tensor("uT", [3 * 64, n_seq], bf16, kind="ExternalOutput")
    s_d = nc.dram_tensor("sT", [3, n_seq], f32, kind="ExternalOutput")

    with tile.TileContext(nc) as tc, ExitStack() as ctx:
        constp = ctx.enter_context(tc.tile_pool(name="const", bufs=1))
        persist = ctx.enter_context(tc.tile_pool(name="persist", bufs=1))
        hpool = ctx.enter_context(tc.tile_pool(name="headsp", bufs=2))
        ypool = ctx.enter_context(tc.tile_pool(name="youtp", bufs=3))
        epool = ctx.enter_context(tc.tile_pool(name="epool", bufs=5))
        rcpool = ctx.enter_context(tc.tile_pool(name="rcp", bufs=2))

        # Transposed weights straight from DRAM: wT[p, d3, e] = W_g[e, 128*d3+p]
        wT = persist.tile([128, 3, 3 * GCOLS], bf16)
        nc.sync.dma_start(out=wT.rearrange("p a b -> p (a b)"), in_=wT_d[:, :])
        # xT[p, d3, n] = x[n, 128*d3 + p]; DMA'd in 4 n-groups so the first
        # qkv chunk can start after ~1/4 of the transfer.
        xT = persist.tile([128, 3, n_seq], bf16)
        ng = n_seq // 4
        for g in range(4):
            nc.sync.dma_start(
                out=xT[:, :, ng * g : ng * (g + 1)],
                in_=xT_d[:, 3 * ng * g : 3 * ng * (g + 1)].rearrange(
                    "p (a b) -> p a b", a=3
                ),
            )

        # Per-head q/k in transposed layout [c, n], bf16, REPLICATED across
        # both 64-partition halves so score matmuls can row-pack in pairs.
        t_q0 = persist.tile([128, n_seq], bf16)
        t_q1 = persist.tile([128, n_seq], bf16)
        t_q2 = persist.tile([128, n_seq], bf16)
        t_k0 = persist.tile([128, n_seq], bf16)
        t_k1 = persist.tile([128, n_seq], bf16)
        t_k2 = persist.tile([128, n_seq], bf16)
        # V in natural layout per n-tile, per head, with a ones column (col 64)
        v_sb = persist.tile([128, nt, 3, 65], bf16)
        ones_nt = constp.tile([128, nt * 3], f32)
        nc.vector.memset(ones_nt, 1.0)
        nc.vector.tensor_copy(
            v_sb[:, :, :, 64:65].rearrange("p t h o -> p (t h o)"), ones_nt
        )

        with (
            tc.tile_pool(name="pqk", bufs=3, space="PSUM") as pqk,
            tc.tile_pool(name="pvp", bufs=2, space="PSUM") as pvp,
        ):
            # ---- qkv per chunk (all bf16) ----
            for j in range(nch):
                # q/k (transposed layout); host lays the W rows out as
                # [q01 (128) | k01 (128) | q2,k2 (128) | v (192)] so each
                # M=128 group is a contiguous lhsT slice. Each 64-row result
                # is copied to the matching partition half of its replicated
                # destination tile.
                for c0, dests in (
                    (0, ((t_q0, 0, 0), (t_q1, 64, 64))),
                    (128, ((t_k0, 0, 0), (t_k1, 64, 64))),
                    (256, ((t_q2, 0, 0), (t_k2, 64, 64))),
                ):
                    ps = pqk.tile([128, 512], f32, tag="pqk")
                    for d3 in range(3):
                        nc.tensor.matmul(
                            ps,
                            wT[:, d3, c0 : c0 + 128],
                            xT[:, d3, 512 * j : 512 * (j + 1)],
                            start=(d3 == 0),
                            stop=(d3 == 2),
                        )
                    for dest, sp, dp in dests:
                        dst_ap = dest[dp : dp + 64, 512 * j : 512 * (j + 1)]
                        if c0 < 256:
                            nc.scalar.copy(dst_ap, ps[sp : sp + 64, :])
                        else:
                            nc.vector.tensor_copy(dst_ap, ps[sp : sp + 64, :])

                # v in natural layout
                for tt in range(4 * j, 4 * j + 4):
                    ps = pvp.tile([128, 192], f32, tag="pv")
                    for d3 in range(3):
                        nc.tensor.matmul(
                            ps,
                            xT[:, d3, 128 * tt : 128 * (tt + 1)],
                            wT[:, d3, 384:576],
                            start=(d3 == 0),
                            stop=(d3 == 2),
                        )
                    nc.vector.tensor_copy(
                        v_sb[:, tt, :, 0:64],
                        ps.rearrange("p (h c) -> p h c", h=3),
                    )

            # replicate each q/k head across the other partition half
            for tq in (t_q0, t_k0):
                nc.sync.dma_start(out=tq[64:128, :], in_=tq[0:64, :])
            for tq in (t_q1, t_k1, t_k2):
                nc.sync.dma_start(out=tq[0:64, :], in_=tq[64:128, :])
            nc.sync.dma_start(out=t_q2[64:128, :], in_=t_q2[0:64, :])

        # ---- attention + projection ----
        # Pipelined: scores for t-pairs into a 2-bank PSUM tile, one exp ACT
        # per pair, PV accumulation trailing. Each head's normalize +
        # projection tail is deferred into the next head's compute so the PE
        # never drains (keeps the HAM clock un-throttled). The tail uses its
        # own 1-bank PSUM pool so it never steals a scores buffer.
        heads_qk = [(t_k0, t_q0), (t_k1, t_q1), (t_k2, t_q2)]
        with (
            tc.tile_pool(name="psc", bufs=2, space="PSUM") as psc,
            tc.tile_pool(name="pop", bufs=2, space="PSUM") as pop,
        ):

            def make_tail(j, h, po_t, headsT):
                # Stash the UNNORMALIZED bf16 head output and its softmax
                # denominator row straight to DRAM; the host does both the
                # normalization and the output projection (one sgemm).
                s_sb = rcpool.tile([65, 512], f32, tag="s_sb", name="s_sb")
                u_sb = rcpool.tile([64, 512], bf16, tag="u_sb", name="u_sb")

                def stash():
                    nc.vector.tensor_copy(u_sb, po_t[0:64, :])
                    nc.scalar.copy(s_sb[64:65, :], po_t[64:65, :])
                    nc.sync.dma_start(
                        out=s_d[h : h + 1, 512 * j : 512 * (j + 1)],
                        in_=s_sb[64:65, :],
                    )

                def ship():
                    nc.sync.dma_start(
                        out=u_d[64 * h : 64 * (h + 1), 512 * j : 512 * (j + 1)],
                        in_=u_sb,
                    )

                # (slot, fn): dispatched once the next head's pair index
                # reaches `slot`; short PE->DVE chains only, so nothing in
                # any engine queue waits long.
                return [(3, stash), (5, ship)]

            PV_DELAY = min(3, nt // 2 - 1)  # pairs the PV matmuls trail by
            OFF_PV_DELAY = PV_DELAY + 2  # extra trail for DVE fast-exp pairs

            # PV queue survives head/chunk boundaries so the PE stream never
            # bunches up (bunched PVs stall the ACT and re-throttle the HAM).
            pv_queue = []
            stop_done = set()  # (po id) whose stop-flagged PV has been emitted
            # deferred 2nd-term + sum ops of DVE fast-exp pairs
            off_queue = []

            def flush_off():
                e_t, ea_t = off_queue.pop(0)
                eb_t = epool.tile([128, 2, 512], bf16, tag="eb", name="eb_t", bufs=6)
                nc.vector.tensor_scalar_add(
                    eb_t.bitcast(i16), ea_t.bitcast(i16), EDELTA
                )
                nc.vector.tensor_add(e_t, ea_t, eb_t)

            def emit_pv(ent):
                _, po_q, h_q, t0, e_t = ent
                for s in range(2):
                    t = t0 + s
                    nc.tensor.matmul(
                        po_q,
                        v_sb[:, t, h_q, :],
                        e_t[:, s, :],
                        start=(t == 0),
                        stop=(t == nt - 1),
                    )
                if t0 == nt - 2:
                    stop_done.add(id(po_q))

            def flush_pv():
                emit_pv(pv_queue.pop(0))

            def pump_pv(g_now):
                # Flush the oldest sufficiently-aged entry; fast-exp pairs
                # age longer so their PV never waits on the deferred sum.
                # (Accumulation order between start and stop is free.)
                for i in range(min(6, len(pv_queue))):
                    g_emit, _, _, t0, e_t = pv_queue[i]
                    if i > 0 and t0 == nt - 2:
                        # never emit the stop-flagged pair out of order
                        return
                    need = OFF_PV_DELAY if (t0 // 2) in OFF_PAIRS else PV_DELAY
                    if g_now - g_emit >= need:
                        emit_pv(pv_queue.pop(i))
                        return

            pending_tail = []
            pending_po_id = None
            for j in range(nch):
                headsT = None
                for h in range(3):
                    kt, qt = heads_qk[h]
                    po_t = pop.tile([65, 512], f32, tag="po")
                    g_head = (3 * j + h) * 16
                    for tp in range(nt // 2):
                        pump_pv(g_head + tp)
                        if off_queue and tp not in OFF_PAIRS:
                            flush_off()
                        t0 = 2 * tp
                        ps = psc.tile([128, 2, 512], f32, tag="ps")
                        for s in range(2):
                            # row-packed pair: s=0 on partitions 0-63,
                            # s=1 on partitions 64-127 (concurrent on PE)
                            t = t0 + s
                            pb = 64 * s
                            nc.tensor.matmul(
                                ps[:, s, :],
                                kt[pb : pb + 64, 128 * t : 128 * (t + 1)],
                                qt[pb : pb + 64, 512 * j : 512 * (j + 1)],
                            )
                        e_t = epool.tile([128, 2, 512], bf16, tag="e_t", bufs=10)
                        if tp in OFF_PAIRS:
                            # DVE fast-exp: affine+round-to-int16 writes a
                            # phase-shifted PL approximation of exp(s*SCALE)
                            # in bf16 bit-space; a second term (bits-63, an
                            # exact int shift that never re-reads PSUM) is
                            # added later so the scores buffer frees as fast
                            # as an ACT exp. Summed max rel err: 1.3%.
                            ea_t = epool.tile([128, 2, 512], bf16, tag="ea", bufs=6)
                            nc.vector.tensor_scalar(
                                out=ea_t.bitcast(i16),
                                in0=ps[:],
                                scalar1=EA,
                                scalar2=EB1,
                                op0=mybir.AluOpType.mult,
                                op1=mybir.AluOpType.add,
                            )
                            off_queue.append((e_t, ea_t))
                        else:
                            nc.scalar.activation(e_t, ps[:], EXP, scale=SCALE)
                        pv_queue.append((g_head + tp, po_t, h, t0, e_t))
                        # tail parts only once their head's accumulation has
                        # fully been emitted (stop PV in the PE stream)
                        while (
                            pending_tail
                            and tp >= pending_tail[0][0]
                            and pending_po_id in stop_done
                        ):
                            pending_tail.pop(0)[1]()
                        if len(pv_queue) > 5:
                            pump_pv(g_head + tp)
                    while pending_tail:
                        if pending_po_id not in stop_done:
                            pump_pv(10 ** 9)
                            continue
                        pending_tail.pop(0)[1]()
                    pending_tail = make_tail(j, h, po_t, headsT)
                    pending_po_id = id(po_t)
            while off_queue:
                flush_off()
            while pv_queue:
                flush_pv()
            for _, part in pending_tail:
                part()

    nc.compile()
    return nc


def shard_inputs(x, W_qkv, W_proj):
    """Full inputs -> per-core input maps (host transposes + bf16 casts)."""
    import ml_dtypes

    bf16 = ml_dtypes.bfloat16
    x = np.asarray(x, dtype=np.float32)
    W_qkv = np.asarray(W_qkv, dtype=np.float32)
    W_proj = np.asarray(W_proj, dtype=np.float32)
    d = W_qkv.shape[1]
    n_seq = x.shape[1]
    in_maps = []
    for c in range(NCORES):
        b, g = divmod(c, 2)
        sl = slice(GCOLS * g, GCOLS * (g + 1))
        w_q = W_qkv[0 * d :][sl]  # [192, 384]
        w_k = W_qkv[1 * d :][sl]
        w_v = W_qkv[2 * d :][sl]
        # Row order [q01 | k01 | q2,k2 | v] so each qkv matmul group is a
        # contiguous 128-col lhsT slice of wT.
        w_g = np.concatenate(
            [w_q[0:128], w_k[0:128], w_q[128:192], w_k[128:192], w_v], axis=0
        )
        # wT[p, d3, e] = W_g[e, 128*d3 + p]  -> [128, 3*576]
        wT = np.ascontiguousarray(
            w_g.T.reshape(3, 128, 3 * GCOLS).transpose(1, 0, 2).reshape(128, -1)
        ).astype(bf16)
        # xT[p, d3, n] = x[b][n, 128*d3 + p] -> [128, 3*n_seq], stored in 4
        # contiguous n-groups of [3, n/4] to match the kernel's grouped DMA.
        xT = (
            x[b].T.reshape(3, 128, n_seq)
            .transpose(1, 0, 2)  # [128, 3, n]
            .reshape(128, 3, 4, n_seq // 4)
            .transpose(0, 2, 1, 3)  # [128, 4, 3, n/4]
            .reshape(128, -1)
        )
        xT = np.ascontiguousarray(xT).astype(bf16)
        in_maps.append({"xT": xT, "wT": wT})
    return in_maps


def combine_outputs(results, W_proj, b_proj, n_seq=NSEQ):
    """Per-core (uT, sT) partials -> full [B, N, D] output.

    uT: [192, n_seq] unnormalized bf16 head outputs; sT: [3, n_seq] softmax
    denominators. Host normalizes, projects (sgemm) and sums head-groups.
    """
    W_proj = np.asarray(W_proj, dtype=np.float32)
    b_proj = np.asarray(b_proj, dtype=np.float32)
    y = np.empty((B, n_seq, D), dtype=np.float32)
    for b in range(B):
        acc = np.zeros((D, n_seq), dtype=np.float32)
        for g in range(2):
            r = results[2 * b + g]
            u = r["uT"].astype(np.float32).reshape(3, 64, n_seq)
            u /= r["sT"][:, None, :]
            wp_g = W_proj[:, GCOLS * g : GCOLS * (g + 1)]  # [D, 192]
            acc += wp_g @ u.reshape(192, n_seq)
        y[b] = acc.T + b_proj
    return y


_NC_CACHE = {}


def kernel(**inputs):
    _ensure_path()
    from concourse.bass_utils import run_bass_kernel_spmd

    x = np.asarray(inputs["x"], dtype=np.float32)
    W_qkv = np.asarray(inputs["W_qkv"], dtype=np.float32)
    W_proj = np.asarray(inputs["W_proj"], dtype=np.float32)
    b_proj = np.asarray(inputs["b_proj"], dtype=np.float32)

    n_seq = x.shape[1]
    if n_seq not in _NC_CACHE:
        _NC_CACHE[n_seq] = build_nc(n_seq)
    nc = _NC_CACHE[n_seq]

    in_maps = shard_inputs(x, W_qkv, W_proj)
    res = run_bass_kernel_spmd(nc, in_maps, core_ids=list(range(NCORES)))
    return combine_outputs(res.results, W_proj, b_proj, n_seq)


if __name__ == "__main__":
    rng = np.random.default_rng(0)
    n = 512
    x = rng.standard_normal((B, n, D), dtype=np.float32)
    wq = (rng.standard_normal((3 * D, D), dtype=np.float32) / np.sqrt(D)).astype(np.float32)
    wp = (rng.standard_normal((D, D), dtype=np.float32) / np.sqrt(D)).astype(np.float32)
    bp = np.zeros(D, np.float32)
    out = kernel(x=x, W_qkv=wq, W_proj=wp, b_proj=bp)
    print(out.shape, out.dtype)
